# revision 15
# baseline (speedup 1.0000x reference)
"""Sharded MHA-with-RoPE Trainium2 kernel (nn_CustomTorchMHASelf).

Contract: kernel(**inputs) takes the FULL unsharded inputs of the
reference (x [2,2048,2048], Wqkv_w [6144,2048], Wqkv_b [6144],
out_w [2048,2048], out_b [2048]) and returns the full [2,2048,2048]
fp32 output, running the compute on 8 NeuronCores.

Sharding: core = b*4 + g handles batch b and head-group g (4 of the 16
heads). Each core computes q/k/v projections for its heads, RoPE,
softmax attention, and its slice of the out-projection; the host sums
the 4 partial outputs per batch and adds out_b.

Device data plane is bf16 (fp32 PSUM accumulation); the host
pre-transposes x and the weight slices into the layouts the TensorE
wants (contraction dim on partitions everywhere).

Schedule (v2): two passes so the softmax exp (ScalarE-bound, ~151us)
hides under TensorE work instead of serializing behind it:
  pass 1: K projection + RoPE and V projection for all tokens.
  pass 2: per 512-token block: Q projection + RoPE, then per head
          scores -> exp -> att@V, then the out-projection for the
          block. Blocks run 3,2,1,0 so pass 1's last x tile is reused.
The softmax denominator is an in-place bf16 tree-add over the 16 key
blocks on VectorE plus a single ones-matmul (instead of 16 full
ones-matmuls per (head, block) on the PE), and the reciprocal uses the
fast custom-DVE approximation.
"""

import math
import os
import sys
import types

import numpy as np
import ml_dtypes

import concourse.bass as bass
import concourse.mybir as mybir
import concourse.tile as tile
from concourse.bass import ds

F32 = mybir.dt.float32
BF16 = mybir.dt.bfloat16
Alu = mybir.AluOpType
Act = mybir.ActivationFunctionType
BF = ml_dtypes.bfloat16

S, E, HTOT, HL, D, P = 2048, 2048, 16, 4, 128, 128

# Filled with the profile exec time (ns) when MHA_TRACE=1; read by test.py.
LAST_EXEC_NS = None


def _install_axon_ntff_shim():
    """Provide antenv.axon_hooks so trace=True can reach the axon NTFF hook."""
    if "antenv.axon_hooks" in sys.modules:
        return
    mod = types.ModuleType("antenv.axon_hooks")
    holder = [None]
    mod.set_axon_ntff_profile_hook = lambda h: holder.__setitem__(0, h)
    mod.get_axon_ntff_profile_hook = lambda: holder[0]
    sys.modules["antenv.axon_hooks"] = mod
    try:
        import antenv
        antenv.axon_hooks = mod
    except ImportError:
        pass
    # boot() ran at interpreter start (sitecustomize), before this module
    # existed, so its NTFF-hook registration was silently skipped. Redo it.
    try:
        from trn_agent_boot.trn_boot import _ntff_profile_via_ctypes
        hook = _ntff_profile_via_ctypes("/opt/axon/libaxon_pjrt.so")
        if hook is not None:
            mod.set_axon_ntff_profile_hook(hook)
    except Exception:
        pass


def _split_multi_waits(nc):
    """Hoist extra sem-waits onto standalone NoOps (one wait per inst).

    This walrus build rejects any instruction carrying more than one
    sync-wait ("Too many sync wait commands"); Tile attaches one wait per
    outstanding semaphore to the consuming instruction. Splitting them
    across same-engine NoOps placed immediately before is equivalent:
    the engine executes serially, so all waits still precede the inst.
    """
    ctr = 0
    for fn in nc.m.functions:
        for blk in fn.blocks:
            out = []
            for inst in blk.instructions:
                si = getattr(inst, "sync_info", None)
                if si is not None and si.on_wait is not None \
                        and len(si.on_wait) > 1:
                    waits = list(si.on_wait)
                    si.on_wait = [waits[-1]]
                    for w in waits[:-1]:
                        ctr += 1
                        nop = mybir.InstNoOp(
                            name=f"I-wsplit-{ctr}", ins=[], outs=[])
                        nop.engine = inst.engine
                        nop.sync_info = mybir.SyncInfo(
                            on_wait=[w], on_update=[])
                        out.append(nop)
                out.append(inst)
            blk.instructions[:] = out


def _build_mha(nc: bass.Bass):
    """Emit the per-core MHA program (one shard) into `nc`."""
    EO = E // P            # contraction subtiles for the projections
    ST = 512               # free-dim tile (one PSUM bank of fp32)
    NS = S // ST
    SB = S // P
    JT = S // P            # key blocks per head
    ET = E // ST
    H = D // 2

    xT = nc.dram_tensor("xT", [E, S], BF16, kind="ExternalInput")
    # k-head columns first (0..HL*D), then q-head columns
    wqkT = nc.dram_tensor("wqkT", [E, 2 * HL * D], BF16, kind="ExternalInput")
    wvT = nc.dram_tensor("wvT", [E, HL * D], BF16, kind="ExternalInput")
    qkb = nc.dram_tensor("qkb", [2 * HL, D], F32, kind="ExternalInput")
    vb = nc.dram_tensor("vb", [HL * D], F32, kind="ExternalInput")
    cosT = nc.dram_tensor("cosT", [D, S], F32, kind="ExternalInput")
    sinT = nc.dram_tensor("sinT", [D, S], F32, kind="ExternalInput")
    owT = nc.dram_tensor("owT", [HL * D, E], BF16, kind="ExternalInput")
    ones = nc.dram_tensor("ones", [P, P], BF16, kind="ExternalInput")
    out = nc.dram_tensor("out", [S, E], F32, kind="ExternalOutput")

    isc = 1.0 / math.sqrt(D)

    from contextlib import ExitStack

    with tile.TileContext(nc) as tc, ExitStack() as stk:
        persist = stk.enter_context(tc.tile_pool(name="persist", bufs=1))
        kT_sb = persist.tile([P, HL, S], BF16)      # k post-RoPE [d, h, s]
        v_sb = persist.tile([P, SB, HL * D], BF16)  # v natural [s%128, s//128, hd]
        ctxT_sb = persist.tile([P, HL, S], BF16)    # [d, h, i]
        ones_sb = persist.tile([P, P], BF16)
        cos_sb = persist.tile([P, S], F32)
        sin_sb = persist.tile([P, S], F32)
        qkb_sb = persist.tile([P, 2 * HL], F32)
        vb_sb = persist.tile([P, HL * D], F32)
        ow_sb = persist.tile([P, HL, E], BF16)
        nc.sync.dma_start(qkb_sb[:], qkb[:].rearrange("c d -> d c"))

        # x stream shared by both passes; rope temps likewise
        xs = stk.enter_context(tc.tile_pool(name="xstream", bufs=2))
        rt = stk.enter_context(tc.tile_pool(name="ropetmp", bufs=1))
        wqp = stk.enter_context(tc.tile_pool(name="wqpool", bufs=1))
        wq_sb = wqp.tile([P, EO, HL * D], BF16)

        psA = stk.enter_context(tc.tile_pool(name="psA", bufs=4, space="PSUM"))
        psS = stk.enter_context(tc.tile_pool(name="psS", bufs=4, space="PSUM"))

        def rope(ps, bias_ap, sl, out_ap):
            # qb = q + bias; rot = half-swap(qb) via DMA (cross-partition
            # moves need DMA); out = qb*cos + rot*sinSW with the rotation
            # sign folded into the host-prepped sin table.
            qb = rt.tile([P, ST], F32, tag="qb")
            nc.vector.tensor_scalar_add(qb[:], ps[:], bias_ap)
            rot = rt.tile([P, ST], F32, tag="rot")
            nc.sync.dma_start(rot[:H], qb[H:])
            nc.sync.dma_start(rot[H:], qb[:H])
            t1 = rt.tile([P, ST], F32, tag="t1")
            t2 = rt.tile([P, ST], F32, tag="t2")
            nc.vector.tensor_tensor(t1[:], qb[:], cos_sb[:, sl], Alu.mult)
            nc.vector.tensor_tensor(t2[:], rot[:], sin_sb[:, sl], Alu.mult)
            nc.vector.tensor_tensor(out_ap, t1[:], t2[:], Alu.add)

        # ---- pass 1: K projection + RoPE, V projection ----
        xt_last = None
        with tc.tile_pool(name="p1w", bufs=1) as p1:
            wk_sb = p1.tile([P, EO, HL * D], BF16)
            wv_sb = p1.tile([P, EO, HL * D], BF16)
            # DMA priority order: the first K matmul group needs all of
            # xt0 + wk, so those go first (interleaved); wv is needed
            # ~20us in, cos/sin/vb/ones only feed VectorE/pass-2 work.
            xt0 = xs.tile([P, EO, ST], BF16, tag="xt", name="xt0")
            for eo in range(EO):
                nc.sync.dma_start(wk_sb[:, eo, :], wqkT[ds(eo * P, P), : HL * D])
                nc.sync.dma_start(xt0[:, eo, :], xT[ds(eo * P, P), ds(0, ST)])
            # x block 1 next: pass 1 is DMA-bound early, and block 1's K
            # matmuls otherwise stall on it
            xt1 = xs.tile([P, EO, ST], BF16, tag="xt", name="xt1")
            for eo in range(EO):
                nc.sync.dma_start(xt1[:, eo, :], xT[ds(eo * P, P), ds(ST, ST)])
            for eo in range(EO):
                nc.sync.dma_start(wv_sb[:, eo, :], wvT[ds(eo * P, P), :])
            nc.sync.dma_start(cos_sb[:], cosT[:])
            nc.sync.dma_start(sin_sb[:], sinT[:])
            nc.sync.dma_start(vb_sb[:], vb[None, :].to_broadcast((P, HL * D)))
            nc.sync.dma_start(ones_sb[:], ones[:])

            for i in range(NS):
                if i == 0:
                    xt = xt0
                elif i == 1:
                    xt = xt1
                else:
                    xt = xs.tile([P, EO, ST], BF16, tag="xt")
                    for eo in range(EO):
                        nc.sync.dma_start(
                            xt[:, eo, :], xT[ds(eo * P, P), ds(i * ST, ST)])
                sl = ds(i * ST, ST)
                for jb in range(HL):       # k head jb
                    ps = psA.tile([P, ST], F32, tag="acc")
                    for eo in range(EO):
                        nc.tensor.matmul(
                            ps[:], wk_sb[:, eo, ds(jb * D, D)], xt[:, eo, :],
                            start=(eo == 0), stop=(eo == EO - 1))
                    rope(ps, qkb_sb[:, jb, None], sl, kT_sb[:, jb, sl])
                for sbl in range(ST // P):
                    sb = i * (ST // P) + sbl
                    ps = psA.tile([P, ST], F32, tag="acc")
                    for eo in range(EO):
                        nc.tensor.matmul(
                            ps[:, : HL * D], xt[:, eo, ds(sbl * P, P)],
                            wv_sb[:, eo, :], start=(eo == 0), stop=(eo == EO - 1))
                    nc.vector.tensor_tensor(
                        v_sb[:, sb, :], ps[:, : HL * D], vb_sb[:], Alu.add)
                if i == 2:
                    # prefetch pass-2 weights; late enough not to delay the
                    # pass-1 x stream, early enough to land before pass 2
                    for eo in range(EO):
                        nc.sync.dma_start(
                            wq_sb[:, eo, :],
                            wqkT[ds(eo * P, P), ds(HL * D, HL * D)])
                    for ho in range(HL):
                        nc.sync.dma_start(ow_sb[:, ho, :], owT[ds(ho * P, P), :])
                if i == NS - 1:
                    xt_last = xt

        # ---- pass 2: per token block: Q + RoPE -> attention -> out proj ----
        # reverse order so block NS-1 reuses pass 1's last x tile
        with tc.tile_pool(name="qpool", bufs=6) as qp, \
             tc.tile_pool(name="attp", bufs=3) as ab, \
             tc.tile_pool(name="denp", bufs=2) as dp, \
             tc.tile_pool(name="ocopy", bufs=2) as oc:
            for i in range(NS - 1, -1, -1):
                if i == NS - 1:
                    xt = xt_last
                else:
                    xt = xs.tile([P, EO, ST], BF16, tag="xt")
                    for eo in range(EO):
                        nc.sync.dma_start(
                            xt[:, eo, :], xT[ds(eo * P, P), ds(i * ST, ST)])
                sl = ds(i * ST, ST)

                def qproj(h):
                    ps = psA.tile([P, ST], F32, tag="acc")
                    for eo in range(EO):
                        nc.tensor.matmul(
                            ps[:], wq_sb[:, eo, ds(h * D, D)], xt[:, eo, :],
                            start=(eo == 0), stop=(eo == EO - 1))
                    qt = qp.tile([P, ST], BF16, tag="qt")
                    rope(ps, qkb_sb[:, HL + h, None], sl, qt[:])
                    return qt

                def scores(h):
                    # scores + exp for head h; runs one head ahead of the
                    # att@v consumers so the ScalarE exp stream is never on
                    # the PE's critical path.
                    qt = qtiles[h]
                    att = ab.tile([P, JT, ST], BF16, tag="att")
                    for jb in range(JT):
                        ps = psS.tile([P, ST], F32, tag="sc")
                        nc.tensor.matmul(
                            ps[:], kT_sb[:, h, ds(jb * P, P)], qt[:],
                            start=True, stop=True)
                        nc.scalar.activation(
                            att[:, jb, :], ps[:], Act.Exp, scale=isc)
                    return att

                def finish(h, att, psc):
                    # denominator ones-matmul + normalize; deferred one head
                    # so the PE queue reaches the ones-matmul well after the
                    # VectorE tree has produced att[:, 0, :].
                    psd = psS.tile([P, ST], F32, tag="sc")
                    nc.tensor.matmul(
                        psd[:], ones_sb[:], att[:, 0, :], start=True, stop=True)
                    rec = dp.tile([P, ST], F32, tag="rec")
                    nc.vector.reciprocal(rec[:], psd[:])
                    nc.vector.tensor_tensor(
                        ctxT_sb[:, h, sl], psc[:], rec[:], Alu.mult)

                qtiles = [qproj(0), qproj(1)]
                atts = [scores(0)]
                pending = None
                for h in range(HL):
                    att = atts[h]
                    if h + 1 < HL:
                        atts.append(scores(h + 1))
                    if h + 2 < HL:
                        qtiles.append(qproj(h + 2))
                    # att @ v, interleaved with the in-place bf16 tree-add
                    # that produces the softmax denominator column-sums in
                    # att[:, 0, :]: att[0:8] is only overwritten after the
                    # matmuls that read it, and att[8:16] is only read.
                    psc = psA.tile([P, ST], F32, tag="acc")
                    for jb in range(8):
                        nc.tensor.matmul(
                            psc[:], v_sb[:, jb, ds(h * D, D)], att[:, jb, :],
                            start=(jb == 0), stop=False)
                    nc.gpsimd.tensor_tensor(
                        att[:, 0:8, :], att[:, 0:8, :], att[:, 8:16, :],
                        Alu.add)
                    for jb in range(8, JT):
                        nc.tensor.matmul(
                            psc[:], v_sb[:, jb, ds(h * D, D)], att[:, jb, :],
                            start=False, stop=(jb == JT - 1))
                    nc.vector.tensor_tensor(
                        att[:, 0:4, :], att[:, 0:4, :], att[:, 4:8, :],
                        Alu.add)
                    nc.vector.tensor_tensor(
                        att[:, 0:2, :], att[:, 0:2, :], att[:, 2:4, :],
                        Alu.add)
                    nc.vector.tensor_tensor(
                        att[:, 0, :], att[:, 0, :], att[:, 1, :], Alu.add)
                    if pending is not None:
                        finish(*pending)
                    pending = (h, att, psc)
                finish(*pending)

                # out projection for this token block
                for sbl in range(ST // P):
                    sb = i * (ST // P) + sbl
                    for et in range(ET):
                        ps = psA.tile([P, ST], F32, tag="acc")
                        for ho in range(HL):
                            nc.tensor.matmul(
                                ps[:], ctxT_sb[:, ho, ds(sb * P, P)],
                                ow_sb[:, ho, ds(et * ST, ST)],
                                start=(ho == 0), stop=(ho == HL - 1))
                        ot = oc.tile([P, ST], F32, tag="ot")
                        nc.any.tensor_copy(ot[:], ps[:])
                        nc.sync.dma_start(
                            out[ds(sb * P, P), ds(et * ST, ST)], ot[:])

    return nc


def _rope_tables():
    inv_freq = 1.0 / (10000.0 ** (np.arange(0, D, 2, dtype=np.float32) / D))
    t = np.arange(S, dtype=np.float32)
    freqs = np.einsum("s,f->sf", t, inv_freq)
    emb = np.concatenate([freqs, freqs], axis=-1)
    cosT = np.cos(emb).astype(np.float32).T.copy()
    sinT = np.sin(emb).astype(np.float32).T.copy()
    # fold the rotate-half sign in: out = qb*cos + halfswap(qb)*sinSW
    sinSW = np.concatenate([-sinT[:D // 2], sinT[D // 2:]], axis=0)
    return cosT, np.ascontiguousarray(sinSW)


def _core_inputs(x, Wqkv_w, Wqkv_b, out_w, b, g, cosT, sinT, xT_bf):
    # k-head columns first, then q-head columns (matches kernel layout)
    k_cols, q_cols, kb_rows, qb_rows = [], [], [], []
    for hl in range(HL):
        h = g * HL + hl
        q_cols.append(Wqkv_w[h * D:(h + 1) * D, :].T)
        k_cols.append(Wqkv_w[E + h * D:E + (h + 1) * D, :].T)
        qb_rows.append(Wqkv_b[h * D:(h + 1) * D])
        kb_rows.append(Wqkv_b[E + h * D:E + (h + 1) * D])
    wqkT = np.ascontiguousarray(
        np.concatenate(k_cols + q_cols, axis=1)).astype(BF)
    qkb = np.stack(kb_rows + qb_rows).astype(np.float32)
    v0 = 2 * E + g * HL * D
    wvT = np.ascontiguousarray(Wqkv_w[v0:v0 + HL * D, :].T).astype(BF)
    vb = Wqkv_b[v0:v0 + HL * D].astype(np.float32)
    owT = np.ascontiguousarray(
        out_w[:, g * HL * D:(g + 1) * HL * D].T).astype(BF)
    return {"xT": xT_bf, "wqkT": wqkT, "wvT": wvT, "qkb": qkb, "vb": vb,
            "cosT": cosT, "sinT": sinT, "owT": owT,
            "ones": np.ones((P, P), BF)}


def kernel(x, Wqkv_w, Wqkv_b, out_w, out_b):
    global LAST_EXEC_NS
    _install_axon_ntff_shim()
    from concourse.bass_utils import run_bass_kernel_spmd

    x = np.asarray(x, dtype=np.float32)
    Wqkv_w = np.asarray(Wqkv_w, dtype=np.float32)
    Wqkv_b = np.asarray(Wqkv_b, dtype=np.float32)
    out_w = np.asarray(out_w, dtype=np.float32)
    out_b = np.asarray(out_b, dtype=np.float32)

    cosT, sinT = _rope_tables()
    xT_bf = [np.ascontiguousarray(x[b].T).astype(BF) for b in range(2)]
    in_maps = []
    for core in range(8):
        b, g = core // 4, core % 4
        in_maps.append(
            _core_inputs(x, Wqkv_w, Wqkv_b, out_w, b, g, cosT, sinT, xT_bf[b]))

    nc = bass.Bass()
    _build_mha(nc)
    _split_multi_waits(nc)

    trace = bool(os.environ.get("MHA_TRACE"))
    if trace:
        # dev-only profiling path; skip the S3 artifact upload
        import concourse.bass_utils as _bu
        _bu.upload_artifacts = lambda tmpdir: tmpdir
    res = run_bass_kernel_spmd(
        nc, in_maps, core_ids=list(range(8)), trace=trace)
    if trace:
        LAST_EXEC_NS = res.exec_time_ns

    out = np.empty((2, S, E), dtype=np.float32)
    for b in range(2):
        acc = res.results[b * 4 + 0]["out"].astype(np.float32).copy()
        for g in range(1, 4):
            acc += res.results[b * 4 + g]["out"]
        out[b] = acc + out_b[None, :]
    return out


# revision 16
# speedup vs baseline: 1.1017x; 1.1017x over previous
"""Sharded MHA-with-RoPE Trainium2 kernel (nn_CustomTorchMHASelf).

Contract: kernel(**inputs) takes the FULL unsharded inputs of the
reference (x [2,2048,2048], Wqkv_w [6144,2048], Wqkv_b [6144],
out_w [2048,2048], out_b [2048]) and returns the full [2,2048,2048]
fp32 output, running the compute on 8 NeuronCores.

Sharding: core = b*4 + g handles batch b and head-group g (4 of the 16
heads). Each core computes q/k/v projections for its heads, RoPE,
softmax attention, and its slice of the out-projection; the host sums
the 4 partial outputs per batch and adds out_b.

Device data plane is bf16 (fp32 PSUM accumulation); the host
pre-transposes x and the weight slices into the layouts the TensorE
wants (contraction dim on partitions everywhere).

Schedule (v2): two passes so the softmax exp (ScalarE-bound, ~151us)
hides under TensorE work instead of serializing behind it:
  pass 1: K projection + RoPE and V projection for all tokens.
  pass 2: per 512-token block: Q projection + RoPE, then per head
          scores -> exp -> att@V, then the out-projection for the
          block. Blocks run 3,2,1,0 so pass 1's last x tile is reused.
The softmax denominator is an in-place bf16 tree-add over the 16 key
blocks on VectorE plus a single ones-matmul (instead of 16 full
ones-matmuls per (head, block) on the PE), and the reciprocal uses the
fast custom-DVE approximation.
"""

import math
import os
import sys
import types

import numpy as np
import ml_dtypes

import concourse.bass as bass
import concourse.mybir as mybir
import concourse.tile as tile
from concourse.bass import ds

F32 = mybir.dt.float32
BF16 = mybir.dt.bfloat16
Alu = mybir.AluOpType
Act = mybir.ActivationFunctionType
BF = ml_dtypes.bfloat16

S, E, HTOT, HL, D, P = 2048, 2048, 16, 4, 128, 128

# Filled with the profile exec time (ns) when MHA_TRACE=1; read by test.py.
LAST_EXEC_NS = None


def _install_axon_ntff_shim():
    """Provide antenv.axon_hooks so trace=True can reach the axon NTFF hook."""
    if "antenv.axon_hooks" in sys.modules:
        return
    mod = types.ModuleType("antenv.axon_hooks")
    holder = [None]
    mod.set_axon_ntff_profile_hook = lambda h: holder.__setitem__(0, h)
    mod.get_axon_ntff_profile_hook = lambda: holder[0]
    sys.modules["antenv.axon_hooks"] = mod
    try:
        import antenv
        antenv.axon_hooks = mod
    except ImportError:
        pass
    # boot() ran at interpreter start (sitecustomize), before this module
    # existed, so its NTFF-hook registration was silently skipped. Redo it.
    try:
        from trn_agent_boot.trn_boot import _ntff_profile_via_ctypes
        hook = _ntff_profile_via_ctypes("/opt/axon/libaxon_pjrt.so")
        if hook is not None:
            mod.set_axon_ntff_profile_hook(hook)
    except Exception:
        pass


def _split_multi_waits(nc):
    """Hoist extra sem-waits onto standalone NoOps (one wait per inst).

    This walrus build rejects any instruction carrying more than one
    sync-wait ("Too many sync wait commands"); Tile attaches one wait per
    outstanding semaphore to the consuming instruction. Splitting them
    across same-engine NoOps placed immediately before is equivalent:
    the engine executes serially, so all waits still precede the inst.
    """
    ctr = 0
    for fn in nc.m.functions:
        for blk in fn.blocks:
            out = []
            for inst in blk.instructions:
                si = getattr(inst, "sync_info", None)
                if si is not None and si.on_wait is not None \
                        and len(si.on_wait) > 1:
                    waits = list(si.on_wait)
                    si.on_wait = [waits[-1]]
                    for w in waits[:-1]:
                        ctr += 1
                        nop = mybir.InstNoOp(
                            name=f"I-wsplit-{ctr}", ins=[], outs=[])
                        nop.engine = inst.engine
                        nop.sync_info = mybir.SyncInfo(
                            on_wait=[w], on_update=[])
                        out.append(nop)
                out.append(inst)
            blk.instructions[:] = out


def _build_mha(nc: bass.Bass):
    """Emit the per-core MHA program (one shard) into `nc`."""
    EO = E // P            # contraction subtiles for the projections
    ST = 512               # free-dim tile (one PSUM bank of fp32)
    NS = S // ST
    SB = S // P
    JT = S // P            # key blocks per head
    ET = E // ST
    H = D // 2

    xT = nc.dram_tensor("xT", [E, S], BF16, kind="ExternalInput")
    # k-head columns first (0..HL*D), then q-head columns
    wqkT = nc.dram_tensor("wqkT", [E, 2 * HL * D], BF16, kind="ExternalInput")
    wvT = nc.dram_tensor("wvT", [E, HL * D], BF16, kind="ExternalInput")
    qkb = nc.dram_tensor("qkb", [2 * HL, D], F32, kind="ExternalInput")
    vb = nc.dram_tensor("vb", [HL * D], F32, kind="ExternalInput")
    cosT = nc.dram_tensor("cosT", [D, S], F32, kind="ExternalInput")
    sinT = nc.dram_tensor("sinT", [D, S], F32, kind="ExternalInput")
    owT = nc.dram_tensor("owT", [HL * D, E], BF16, kind="ExternalInput")
    ones = nc.dram_tensor("ones", [P, P], BF16, kind="ExternalInput")
    out = nc.dram_tensor("out", [S, E], F32, kind="ExternalOutput")

    isc = 1.0 / math.sqrt(D)

    from contextlib import ExitStack

    with tile.TileContext(nc) as tc, ExitStack() as stk:
        persist = stk.enter_context(tc.tile_pool(name="persist", bufs=1))
        kT_sb = persist.tile([P, HL, S], BF16)      # k post-RoPE [d, h, s]
        v_sb = persist.tile([P, SB, HL * D], BF16)  # v natural [s%128, s//128, hd]
        ctxT_sb = persist.tile([P, HL, S], BF16)    # [d, h, i]
        ones_sb = persist.tile([P, P], BF16)
        cos_sb = persist.tile([P, S], F32)
        sin_sb = persist.tile([P, S], F32)
        qkb_sb = persist.tile([P, 2 * HL], F32)
        vb_sb = persist.tile([P, HL * D], F32)
        ow_sb = persist.tile([P, HL, E], BF16)
        nc.sync.dma_start(qkb_sb[:], qkb[:].rearrange("c d -> d c"))

        # x stream shared by both passes; rope temps likewise
        xs = stk.enter_context(tc.tile_pool(name="xstream", bufs=2))
        rt = stk.enter_context(tc.tile_pool(name="ropetmp", bufs=1))
        wqp = stk.enter_context(tc.tile_pool(name="wqpool", bufs=1))
        wq_sb = wqp.tile([P, EO, HL * D], BF16)

        psA = stk.enter_context(tc.tile_pool(name="psA", bufs=4, space="PSUM"))
        psS = stk.enter_context(tc.tile_pool(name="psS", bufs=4, space="PSUM"))

        def rope(ps, bias_ap, sl, out_ap):
            # qb = q + bias; rot = half-swap(qb) via DMA (cross-partition
            # moves need DMA); out = qb*cos + rot*sinSW with the rotation
            # sign folded into the host-prepped sin table.
            qb = rt.tile([P, ST], F32, tag="qb")
            nc.vector.tensor_scalar_add(qb[:], ps[:], bias_ap)
            rot = rt.tile([P, ST], F32, tag="rot")
            nc.sync.dma_start(rot[:H], qb[H:])
            nc.sync.dma_start(rot[H:], qb[:H])
            t1 = rt.tile([P, ST], F32, tag="t1")
            t2 = rt.tile([P, ST], F32, tag="t2")
            nc.vector.tensor_tensor(t1[:], qb[:], cos_sb[:, sl], Alu.mult)
            nc.vector.tensor_tensor(t2[:], rot[:], sin_sb[:, sl], Alu.mult)
            nc.vector.tensor_tensor(out_ap, t1[:], t2[:], Alu.add)

        # ---- pass 1: K projection + RoPE, V projection ----
        xt_last = None
        with tc.tile_pool(name="p1w", bufs=1) as p1:
            wk_sb = p1.tile([P, EO, HL * D], BF16)
            wv_sb = p1.tile([P, EO, HL * D], BF16)
            # DMA priority order: the first K matmul group needs all of
            # xt0 + wk, so those go first (interleaved); wv is needed
            # ~20us in, cos/sin/vb/ones only feed VectorE/pass-2 work.
            xt0 = xs.tile([P, EO, ST], BF16, tag="xt", name="xt0")
            for eo in range(EO):
                nc.sync.dma_start(wk_sb[:, eo, :], wqkT[ds(eo * P, P), : HL * D])
                nc.sync.dma_start(xt0[:, eo, :], xT[ds(eo * P, P), ds(0, ST)])
            # x block 1 next: pass 1 is DMA-bound early, and block 1's K
            # matmuls otherwise stall on it
            xt1 = xs.tile([P, EO, ST], BF16, tag="xt", name="xt1")
            for eo in range(EO):
                nc.sync.dma_start(xt1[:, eo, :], xT[ds(eo * P, P), ds(ST, ST)])
            for eo in range(EO):
                nc.sync.dma_start(wv_sb[:, eo, :], wvT[ds(eo * P, P), :])
            nc.sync.dma_start(cos_sb[:], cosT[:])
            nc.sync.dma_start(sin_sb[:], sinT[:])
            nc.sync.dma_start(vb_sb[:], vb[None, :].to_broadcast((P, HL * D)))
            nc.sync.dma_start(ones_sb[:], ones[:])

            for i in range(NS):
                if i == 0:
                    xt = xt0
                elif i == 1:
                    xt = xt1
                else:
                    xt = xs.tile([P, EO, ST], BF16, tag="xt")
                    for eo in range(EO):
                        nc.sync.dma_start(
                            xt[:, eo, :], xT[ds(eo * P, P), ds(i * ST, ST)])
                sl = ds(i * ST, ST)
                for jb in range(HL):       # k head jb
                    ps = psA.tile([P, ST], F32, tag="acc")
                    for eo in range(EO):
                        nc.tensor.matmul(
                            ps[:], wk_sb[:, eo, ds(jb * D, D)], xt[:, eo, :],
                            start=(eo == 0), stop=(eo == EO - 1))
                    rope(ps, qkb_sb[:, jb, None], sl, kT_sb[:, jb, sl])
                for sbl in range(ST // P):
                    sb = i * (ST // P) + sbl
                    ps = psA.tile([P, ST], F32, tag="acc")
                    for eo in range(EO):
                        nc.tensor.matmul(
                            ps[:, : HL * D], xt[:, eo, ds(sbl * P, P)],
                            wv_sb[:, eo, :], start=(eo == 0), stop=(eo == EO - 1))
                    nc.vector.tensor_tensor(
                        v_sb[:, sb, :], ps[:, : HL * D], vb_sb[:], Alu.add)
                if i == 2:
                    # prefetch pass-2 weights; late enough not to delay the
                    # pass-1 x stream, early enough to land before pass 2
                    for eo in range(EO):
                        nc.sync.dma_start(
                            wq_sb[:, eo, :],
                            wqkT[ds(eo * P, P), ds(HL * D, HL * D)])
                    for ho in range(HL):
                        nc.sync.dma_start(ow_sb[:, ho, :], owT[ds(ho * P, P), :])
                if i == NS - 1:
                    xt_last = xt

        # ---- pass 2: per token block: Q + RoPE -> attention -> out proj ----
        # reverse order so block NS-1 reuses pass 1's last x tile
        with tc.tile_pool(name="qpool", bufs=6) as qp, \
             tc.tile_pool(name="attp", bufs=3) as ab, \
             tc.tile_pool(name="denp", bufs=2) as dp, \
             tc.tile_pool(name="ocopy", bufs=2) as oc:
            for i in range(NS - 1, -1, -1):
                if i == NS - 1:
                    xt = xt_last
                else:
                    xt = xs.tile([P, EO, ST], BF16, tag="xt")
                    for eo in range(EO):
                        nc.sync.dma_start(
                            xt[:, eo, :], xT[ds(eo * P, P), ds(i * ST, ST)])
                sl = ds(i * ST, ST)

                def qproj(h):
                    ps = psA.tile([P, ST], F32, tag="acc")
                    for eo in range(EO):
                        nc.tensor.matmul(
                            ps[:], wq_sb[:, eo, ds(h * D, D)], xt[:, eo, :],
                            start=(eo == 0), stop=(eo == EO - 1))
                    qt = qp.tile([P, ST], BF16, tag="qt")
                    rope(ps, qkb_sb[:, HL + h, None], sl, qt[:])
                    return qt

                def scores(h):
                    # scores + exp for head h; runs one head ahead of the
                    # att@v consumers so the ScalarE exp stream is never on
                    # the PE's critical path.
                    qt = qtiles[h]
                    att = ab.tile([P, JT, ST], BF16, tag="att")
                    for jb in range(JT):
                        ps = psS.tile([P, ST], F32, tag="sc")
                        nc.tensor.matmul(
                            ps[:], kT_sb[:, h, ds(jb * P, P)], qt[:],
                            start=True, stop=True)
                        nc.scalar.activation(
                            att[:, jb, :], ps[:], Act.Exp, scale=isc)
                    return att

                def finish(h, att, psc):
                    # denominator ones-matmul + normalize; deferred one head
                    # so the PE queue reaches the ones-matmul well after the
                    # VectorE tree has produced att[:, 0, :].
                    psd = psS.tile([P, ST], F32, tag="sc")
                    nc.tensor.matmul(
                        psd[:], ones_sb[:], att[:, 0, :], start=True, stop=True)
                    rec = dp.tile([P, ST], F32, tag="rec")
                    nc.vector.reciprocal(rec[:], psd[:])
                    nc.vector.tensor_tensor(
                        ctxT_sb[:, h, sl], psc[:], rec[:], Alu.mult)

                qtiles = [qproj(0), qproj(1)]
                atts = [scores(0)]
                pending = None
                for h in range(HL):
                    att = atts[h]
                    if h + 1 < HL:
                        atts.append(scores(h + 1))
                    if h + 2 < HL:
                        qtiles.append(qproj(h + 2))
                    # att @ v, interleaved with the in-place bf16 tree-add
                    # that produces the softmax denominator column-sums in
                    # att[:, 0, :]: att[0:8] is only overwritten after the
                    # matmuls that read it, and att[8:16] is only read.
                    psc = psA.tile([P, ST], F32, tag="acc")
                    for jb in range(8):
                        nc.tensor.matmul(
                            psc[:], v_sb[:, jb, ds(h * D, D)], att[:, jb, :],
                            start=(jb == 0), stop=False)
                    nc.vector.tensor_tensor(
                        att[:, 0:8, :], att[:, 0:8, :], att[:, 8:16, :],
                        Alu.add)
                    for jb in range(8, JT):
                        nc.tensor.matmul(
                            psc[:], v_sb[:, jb, ds(h * D, D)], att[:, jb, :],
                            start=False, stop=(jb == JT - 1))
                    nc.vector.tensor_tensor(
                        att[:, 0:4, :], att[:, 0:4, :], att[:, 4:8, :],
                        Alu.add)
                    nc.vector.tensor_tensor(
                        att[:, 0:2, :], att[:, 0:2, :], att[:, 2:4, :],
                        Alu.add)
                    nc.vector.tensor_tensor(
                        att[:, 0, :], att[:, 0, :], att[:, 1, :], Alu.add)
                    if pending is not None:
                        finish(*pending)
                    pending = (h, att, psc)
                finish(*pending)

                # out projection for this token block
                for sbl in range(ST // P):
                    sb = i * (ST // P) + sbl
                    for et in range(ET):
                        ps = psA.tile([P, ST], F32, tag="acc")
                        for ho in range(HL):
                            nc.tensor.matmul(
                                ps[:], ctxT_sb[:, ho, ds(sb * P, P)],
                                ow_sb[:, ho, ds(et * ST, ST)],
                                start=(ho == 0), stop=(ho == HL - 1))
                        ot = oc.tile([P, ST], F32, tag="ot")
                        nc.any.tensor_copy(ot[:], ps[:])
                        nc.sync.dma_start(
                            out[ds(sb * P, P), ds(et * ST, ST)], ot[:])

    return nc


def _rope_tables():
    inv_freq = 1.0 / (10000.0 ** (np.arange(0, D, 2, dtype=np.float32) / D))
    t = np.arange(S, dtype=np.float32)
    freqs = np.einsum("s,f->sf", t, inv_freq)
    emb = np.concatenate([freqs, freqs], axis=-1)
    cosT = np.cos(emb).astype(np.float32).T.copy()
    sinT = np.sin(emb).astype(np.float32).T.copy()
    # fold the rotate-half sign in: out = qb*cos + halfswap(qb)*sinSW
    sinSW = np.concatenate([-sinT[:D // 2], sinT[D // 2:]], axis=0)
    return cosT, np.ascontiguousarray(sinSW)


def _core_inputs(x, Wqkv_w, Wqkv_b, out_w, b, g, cosT, sinT, xT_bf):
    # k-head columns first, then q-head columns (matches kernel layout)
    k_cols, q_cols, kb_rows, qb_rows = [], [], [], []
    for hl in range(HL):
        h = g * HL + hl
        q_cols.append(Wqkv_w[h * D:(h + 1) * D, :].T)
        k_cols.append(Wqkv_w[E + h * D:E + (h + 1) * D, :].T)
        qb_rows.append(Wqkv_b[h * D:(h + 1) * D])
        kb_rows.append(Wqkv_b[E + h * D:E + (h + 1) * D])
    wqkT = np.ascontiguousarray(
        np.concatenate(k_cols + q_cols, axis=1)).astype(BF)
    qkb = np.stack(kb_rows + qb_rows).astype(np.float32)
    v0 = 2 * E + g * HL * D
    wvT = np.ascontiguousarray(Wqkv_w[v0:v0 + HL * D, :].T).astype(BF)
    vb = Wqkv_b[v0:v0 + HL * D].astype(np.float32)
    owT = np.ascontiguousarray(
        out_w[:, g * HL * D:(g + 1) * HL * D].T).astype(BF)
    return {"xT": xT_bf, "wqkT": wqkT, "wvT": wvT, "qkb": qkb, "vb": vb,
            "cosT": cosT, "sinT": sinT, "owT": owT,
            "ones": np.ones((P, P), BF)}


def kernel(x, Wqkv_w, Wqkv_b, out_w, out_b):
    global LAST_EXEC_NS
    _install_axon_ntff_shim()
    from concourse.bass_utils import run_bass_kernel_spmd

    x = np.asarray(x, dtype=np.float32)
    Wqkv_w = np.asarray(Wqkv_w, dtype=np.float32)
    Wqkv_b = np.asarray(Wqkv_b, dtype=np.float32)
    out_w = np.asarray(out_w, dtype=np.float32)
    out_b = np.asarray(out_b, dtype=np.float32)

    cosT, sinT = _rope_tables()
    xT_bf = [np.ascontiguousarray(x[b].T).astype(BF) for b in range(2)]
    in_maps = []
    for core in range(8):
        b, g = core // 4, core % 4
        in_maps.append(
            _core_inputs(x, Wqkv_w, Wqkv_b, out_w, b, g, cosT, sinT, xT_bf[b]))

    nc = bass.Bass()
    _build_mha(nc)
    _split_multi_waits(nc)

    trace = bool(os.environ.get("MHA_TRACE"))
    if trace:
        # dev-only profiling path; skip the S3 artifact upload
        import concourse.bass_utils as _bu
        _bu.upload_artifacts = lambda tmpdir: tmpdir
    res = run_bass_kernel_spmd(
        nc, in_maps, core_ids=list(range(8)), trace=trace)
    if trace:
        LAST_EXEC_NS = res.exec_time_ns

    out = np.empty((2, S, E), dtype=np.float32)
    for b in range(2):
        acc = res.results[b * 4 + 0]["out"].astype(np.float32).copy()
        for g in range(1, 4):
            acc += res.results[b * 4 + g]["out"]
        out[b] = acc + out_b[None, :]
    return out


# revision 17
# speedup vs baseline: 1.5494x; 1.4064x over previous
"""Sharded MHA-with-RoPE Trainium2 kernel (nn_CustomTorchMHASelf).

Contract: kernel(**inputs) takes the FULL unsharded inputs of the
reference (x [2,2048,2048], Wqkv_w [6144,2048], Wqkv_b [6144],
out_w [2048,2048], out_b [2048]) and returns the full [2,2048,2048]
fp32 output, running the compute on 8 NeuronCores.

Sharding: core = b*4 + g handles batch b and head-group g (4 of the 16
heads). Each core computes q/k/v projections for its heads, RoPE,
softmax attention, and its slice of the out-projection; the host sums
the 4 partial outputs per batch and adds out_b.

Device data plane is bf16 (fp32 PSUM accumulation); the host
pre-transposes x and the weight slices into the layouts the TensorE
wants (contraction dim on partitions everywhere).

Schedule (v8): pass 1 computes K+RoPE and V for all tokens; pass 2
walks 512-token blocks (reverse order, reusing pass 1's last x tile)
with a software-pipelined head loop:
  - scores/exp for head h+1 are interleaved matmul-by-matmul with
    att@V for head h, so the PSUM-bank recycling that paces scores on
    the exp stream is hidden under att@V work;
  - the softmax denominator is an in-place bf16 tree-add over the 16
    key blocks on VectorE plus ONE ones-matmul (vs 16 full PE
    ones-matmuls), deferred one head so the PE never waits on the tree;
  - 1/denominator = Exp(-Ln(d)) on ScalarE (both functions live in the
    same activation table, so no table reloads) because DVE reciprocal
    is slow and custom-DVE ops don't compile on this toolchain;
  - the block's out-projection is deferred one block and interleaved
    into the next block's scores stream, keeping its DMA off the tail.
"""

import math
import os
import sys
import types

import numpy as np
import ml_dtypes

import concourse.bass as bass
import concourse.mybir as mybir
import concourse.tile as tile
from concourse.bass import ds

F32 = mybir.dt.float32
BF16 = mybir.dt.bfloat16
Alu = mybir.AluOpType
Act = mybir.ActivationFunctionType
BF = ml_dtypes.bfloat16

S, E, HTOT, HL, D, P = 2048, 2048, 16, 4, 128, 128

# Filled with the profile exec time (ns) when MHA_TRACE=1; read by test.py.
LAST_EXEC_NS = None


def _install_axon_ntff_shim():
    """Provide antenv.axon_hooks so trace=True can reach the axon NTFF hook."""
    if "antenv.axon_hooks" in sys.modules:
        return
    mod = types.ModuleType("antenv.axon_hooks")
    holder = [None]
    mod.set_axon_ntff_profile_hook = lambda h: holder.__setitem__(0, h)
    mod.get_axon_ntff_profile_hook = lambda: holder[0]
    sys.modules["antenv.axon_hooks"] = mod
    try:
        import antenv
        antenv.axon_hooks = mod
    except ImportError:
        pass
    # boot() ran at interpreter start (sitecustomize), before this module
    # existed, so its NTFF-hook registration was silently skipped. Redo it.
    try:
        from trn_agent_boot.trn_boot import _ntff_profile_via_ctypes
        hook = _ntff_profile_via_ctypes("/opt/axon/libaxon_pjrt.so")
        if hook is not None:
            mod.set_axon_ntff_profile_hook(hook)
    except Exception:
        pass


def _split_multi_waits(nc):
    """Hoist extra sem-waits onto standalone NoOps (one wait per inst).

    This walrus build rejects any instruction carrying more than one
    sync-wait ("Too many sync wait commands"); Tile attaches one wait per
    outstanding semaphore to the consuming instruction. Splitting them
    across same-engine NoOps placed immediately before is equivalent:
    the engine executes serially, so all waits still precede the inst.
    """
    ctr = 0
    for fn in nc.m.functions:
        for blk in fn.blocks:
            out = []
            for inst in blk.instructions:
                si = getattr(inst, "sync_info", None)
                if si is not None and si.on_wait is not None \
                        and len(si.on_wait) > 1:
                    waits = list(si.on_wait)
                    si.on_wait = [waits[-1]]
                    for w in waits[:-1]:
                        ctr += 1
                        nop = mybir.InstNoOp(
                            name=f"I-wsplit-{ctr}", ins=[], outs=[])
                        nop.engine = inst.engine
                        nop.sync_info = mybir.SyncInfo(
                            on_wait=[w], on_update=[])
                        out.append(nop)
                out.append(inst)
            blk.instructions[:] = out


def _build_mha(nc: bass.Bass):
    """Emit the per-core MHA program (one shard) into `nc`."""
    EO = E // P            # contraction subtiles for the projections
    ST = 512               # free-dim tile (one PSUM bank of fp32)
    NS = S // ST
    SB = S // P
    JT = S // P            # key blocks per head
    ET = E // ST
    H = D // 2

    xT = nc.dram_tensor("xT", [E, S], BF16, kind="ExternalInput")
    # k-head columns first (0..HL*D), then q-head columns
    wqkT = nc.dram_tensor("wqkT", [E, 2 * HL * D], BF16, kind="ExternalInput")
    wvT = nc.dram_tensor("wvT", [E, HL * D], BF16, kind="ExternalInput")
    qkb = nc.dram_tensor("qkb", [2 * HL, D], F32, kind="ExternalInput")
    vb = nc.dram_tensor("vb", [HL * D], F32, kind="ExternalInput")
    cosT = nc.dram_tensor("cosT", [D, S], BF16, kind="ExternalInput")
    sinT = nc.dram_tensor("sinT", [D, S], BF16, kind="ExternalInput")
    owT = nc.dram_tensor("owT", [HL * D, E], BF16, kind="ExternalInput")
    ones = nc.dram_tensor("ones", [P, P], BF16, kind="ExternalInput")
    out = nc.dram_tensor("out", [S, E], BF16, kind="ExternalOutput")

    isc = 1.0 / math.sqrt(D)

    from contextlib import ExitStack

    with tile.TileContext(nc) as tc, ExitStack() as stk:
        persist = stk.enter_context(tc.tile_pool(name="persist", bufs=1))
        kT_sb = persist.tile([P, HL, S], BF16)      # k post-RoPE [d, h, s]
        v_sb = persist.tile([P, SB, HL * D], BF16)  # v natural [s%128, s//128, hd]
        ctxT_sb = persist.tile([P, HL, S], BF16)    # [d, h, i]
        ones_sb = persist.tile([P, P], BF16)
        cos_sb = persist.tile([P, S], BF16)
        sin_sb = persist.tile([P, S], BF16)
        qkb_sb = persist.tile([P, 2 * HL], F32)
        vb_sb = persist.tile([P, HL * D], F32)
        ow_sb = persist.tile([P, HL, E], BF16)
        nc.sync.dma_start(qkb_sb[:], qkb[:].rearrange("c d -> d c"))

        # x stream shared by both passes; rope temps likewise.  qb/rot are
        # still being read (by the swap DMAs / mults) when the next rope
        # starts, so they get 2 bufs; t1/t2 are consumed immediately by the
        # in-order VectorE queue, so 1 buf suffices.
        xs = stk.enter_context(tc.tile_pool(name="xstream", bufs=2))
        rta = stk.enter_context(tc.tile_pool(name="ropetmpa", bufs=2))
        rtb = stk.enter_context(tc.tile_pool(name="ropetmpb", bufs=1))
        wqp = stk.enter_context(tc.tile_pool(name="wqpool", bufs=1))
        wq_sb = wqp.tile([P, EO, HL * D], BF16)

        psA = stk.enter_context(tc.tile_pool(name="psA", bufs=4, space="PSUM"))
        psS = stk.enter_context(tc.tile_pool(name="psS", bufs=3, space="PSUM"))
        psD = stk.enter_context(tc.tile_pool(name="psD", bufs=1, space="PSUM"))

        def rope(ps, bias_ap, sl, out_ap):
            # qb = q + bias; rot = half-swap(qb) via DMA (cross-partition
            # moves need DMA); out = qb*cos + rot*sinSW with the rotation
            # sign folded into the host-prepped sin table.
            qb = rta.tile([P, ST], F32, tag="qb")
            nc.vector.tensor_scalar_add(qb[:], ps[:], bias_ap)
            rot = rta.tile([P, ST], F32, tag="rot")
            nc.sync.dma_start(rot[:H], qb[H:])
            nc.sync.dma_start(rot[H:], qb[:H])
            t1 = rtb.tile([P, ST], F32, tag="t1")
            t2 = rtb.tile([P, ST], F32, tag="t2")
            nc.vector.tensor_tensor(t1[:], qb[:], cos_sb[:, sl], Alu.mult)
            nc.vector.tensor_tensor(t2[:], rot[:], sin_sb[:, sl], Alu.mult)
            nc.vector.tensor_tensor(out_ap, t1[:], t2[:], Alu.add)

        # ---- pass 1: K projection + RoPE, V projection ----
        xt_last = None
        with tc.tile_pool(name="p1w", bufs=1) as p1:
            wk_sb = p1.tile([P, EO, HL * D], BF16)
            wv_sb = p1.tile([P, EO, HL * D], BF16)
            # DMA priority order: the first K matmul group needs all of
            # xt0 + wk; cos/sin (bf16, 1MB) unblock the first RoPEs; then
            # x block 1, wv (needed ~25us in), and the small tables.
            xt0 = xs.tile([P, EO, ST], BF16, tag="xt", name="xt0")
            for eo in range(EO):
                nc.sync.dma_start(wk_sb[:, eo, :], wqkT[ds(eo * P, P), : HL * D])
                nc.sync.dma_start(xt0[:, eo, :], xT[ds(eo * P, P), ds(0, ST)])
            nc.sync.dma_start(cos_sb[:], cosT[:])
            nc.sync.dma_start(sin_sb[:], sinT[:])
            xt1 = xs.tile([P, EO, ST], BF16, tag="xt", name="xt1")
            for eo in range(EO):
                nc.sync.dma_start(xt1[:, eo, :], xT[ds(eo * P, P), ds(ST, ST)])
            for eo in range(EO):
                nc.sync.dma_start(wv_sb[:, eo, :], wvT[ds(eo * P, P), :])
            nc.sync.dma_start(vb_sb[:], vb[None, :].to_broadcast((P, HL * D)))
            nc.sync.dma_start(ones_sb[:], ones[:])

            for i in range(NS):
                if i == 0:
                    xt = xt0
                elif i == 1:
                    xt = xt1
                else:
                    xt = xs.tile([P, EO, ST], BF16, tag="xt")
                    for eo in range(EO):
                        nc.sync.dma_start(
                            xt[:, eo, :], xT[ds(eo * P, P), ds(i * ST, ST)])
                sl = ds(i * ST, ST)
                for jb in range(HL):       # k head jb
                    ps = psA.tile([P, ST], F32, tag="acc")
                    for eo in range(EO):
                        nc.tensor.matmul(
                            ps[:], wk_sb[:, eo, ds(jb * D, D)], xt[:, eo, :],
                            start=(eo == 0), stop=(eo == EO - 1))
                    rope(ps, qkb_sb[:, jb, None], sl, kT_sb[:, jb, sl])
                for sbl in range(ST // P):
                    sb = i * (ST // P) + sbl
                    ps = psA.tile([P, ST], F32, tag="acc")
                    for eo in range(EO):
                        nc.tensor.matmul(
                            ps[:, : HL * D], xt[:, eo, ds(sbl * P, P)],
                            wv_sb[:, eo, :], start=(eo == 0), stop=(eo == EO - 1))
                    nc.vector.tensor_tensor(
                        v_sb[:, sb, :], ps[:, : HL * D], vb_sb[:], Alu.add)
                if i == 2:
                    # prefetch pass-2 weights; late enough not to delay the
                    # pass-1 x stream, early enough to land before pass 2
                    for eo in range(EO):
                        nc.sync.dma_start(
                            wq_sb[:, eo, :],
                            wqkT[ds(eo * P, P), ds(HL * D, HL * D)])
                    for ho in range(HL):
                        nc.sync.dma_start(ow_sb[:, ho, :], owT[ds(ho * P, P), :])
                if i == NS - 1:
                    xt_last = xt

        # ---- pass 2: per token block: Q + RoPE -> attention [-> out proj] --
        # reverse order so block NS-1 reuses pass 1's last x tile
        with tc.tile_pool(name="qpool", bufs=6) as qp, \
             tc.tile_pool(name="attp", bufs=3) as ab, \
             tc.tile_pool(name="denp", bufs=2) as dp, \
             tc.tile_pool(name="ocopy", bufs=2) as oc:

            def qproj(h, xt, sl):
                ps = psA.tile([P, ST], F32, tag="acc")
                for eo in range(EO):
                    nc.tensor.matmul(
                        ps[:], wq_sb[:, eo, ds(h * D, D)], xt[:, eo, :],
                        start=(eo == 0), stop=(eo == EO - 1))
                qt = qp.tile([P, ST], BF16, tag="qt")
                rope(ps, qkb_sb[:, HL + h, None], sl, qt[:])
                return qt

            def cblock_tile(ci, jb):
                # one out-projection tile (of 16) for token block ci
                sbl, et = jb // ET, jb % ET
                sb = ci * (ST // P) + sbl
                ps = psA.tile([P, ST], F32, tag="acc")
                for ho in range(HL):
                    nc.tensor.matmul(
                        ps[:], ctxT_sb[:, ho, ds(sb * P, P)],
                        ow_sb[:, ho, ds(et * ST, ST)],
                        start=(ho == 0), stop=(ho == HL - 1))
                ot = oc.tile([P, ST], BF16, tag="ot")
                nc.vector.tensor_copy(ot[:], ps[:])
                nc.sync.dma_start(out[ds(sb * P, P), ds(et * ST, ST)], ot[:])

            def finish(h, att, psc, sl):
                # denominator ones-matmul + normalize; deferred one head so
                # the PE reaches the ones-matmul well after the VectorE tree
                # produced att[:, 0, :].  1/d = Exp(-Ln(d)) on ScalarE (ln
                # and exp share an activation table -> no table reloads).
                psd = psD.tile([P, ST], F32, tag="d")
                nc.tensor.matmul(
                    psd[:], ones_sb[:], att[:, 0, :], start=True, stop=True)
                lnd = dp.tile([P, ST], F32, tag="lnd")
                nc.scalar.activation(lnd[:], psd[:], Act.Ln)
                rec = dp.tile([P, ST], F32, tag="rec")
                nc.scalar.activation(rec[:], lnd[:], Act.Exp, scale=-1.0)
                nc.vector.tensor_tensor(
                    ctxT_sb[:, h, sl], psc[:], rec[:], Alu.mult)

            prev_ci = None
            order = list(range(NS - 1, -1, -1))
            xt_next = None
            for idx, i in enumerate(order):
                xt = xt_last if i == NS - 1 else xt_next
                sl = ds(i * ST, ST)
                # prefetch the NEXT block's x now; the DMA has the whole
                # block (~50us) to land instead of stalling at its start
                if idx + 1 < len(order):
                    nxt = order[idx + 1]
                    xt_next = xs.tile([P, EO, ST], BF16, tag="xt")
                    for eo in range(EO):
                        nc.sync.dma_start(
                            xt_next[:, eo, :],
                            xT[ds(eo * P, P), ds(nxt * ST, ST)])

                qtiles = [qproj(0, xt, sl), qproj(1, xt, sl)]
                # scores+exp for head 0, interleaved with the deferred
                # out-projection of the previous block
                att0 = ab.tile([P, JT, ST], BF16, tag="att")
                for jb in range(JT):
                    ps = psS.tile([P, ST], F32, tag="sc")
                    nc.tensor.matmul(
                        ps[:], kT_sb[:, 0, ds(jb * P, P)], qtiles[0][:],
                        start=True, stop=True)
                    nc.scalar.activation(
                        att0[:, jb, :], ps[:], Act.Exp, scale=isc)
                    if prev_ci is not None:
                        cblock_tile(prev_ci, jb)

                atts = [att0]
                pending = None
                for h in range(HL):
                    att = atts[h]
                    psc = psA.tile([P, ST], F32, tag="acc")
                    if h + 1 < HL:
                        # pairwise interleave scores(h+1) with att@V(h): the
                        # scores matmuls are paced by exp freeing PSUM banks,
                        # and the att@V matmuls fill those gaps.
                        qt1 = qtiles[h + 1]
                        attn = ab.tile([P, JT, ST], BF16, tag="att")
                        atts.append(attn)
                        for jb in range(JT):
                            ps = psS.tile([P, ST], F32, tag="sc")
                            nc.tensor.matmul(
                                ps[:], kT_sb[:, h + 1, ds(jb * P, P)], qt1[:],
                                start=True, stop=True)
                            nc.scalar.activation(
                                attn[:, jb, :], ps[:], Act.Exp, scale=isc)
                            nc.tensor.matmul(
                                psc[:], v_sb[:, jb, ds(h * D, D)],
                                att[:, jb, :],
                                start=(jb == 0), stop=(jb == JT - 1))
                            if jb == 7:
                                # first tree level: att[0:8] += att[8:16],
                                # after the matmuls that read att[0:8]
                                nc.vector.tensor_tensor(
                                    att[:, 0:8, :], att[:, 0:8, :],
                                    att[:, 8:16, :], Alu.add)
                    else:
                        for jb in range(8):
                            nc.tensor.matmul(
                                psc[:], v_sb[:, jb, ds(h * D, D)],
                                att[:, jb, :], start=(jb == 0), stop=False)
                        nc.vector.tensor_tensor(
                            att[:, 0:8, :], att[:, 0:8, :], att[:, 8:16, :],
                            Alu.add)
                        for jb in range(8, JT):
                            nc.tensor.matmul(
                                psc[:], v_sb[:, jb, ds(h * D, D)],
                                att[:, jb, :],
                                start=False, stop=(jb == JT - 1))
                    if h + 2 < HL:
                        qtiles.append(qproj(h + 2, xt, sl))
                    nc.vector.tensor_tensor(
                        att[:, 0:4, :], att[:, 0:4, :], att[:, 4:8, :],
                        Alu.add)
                    nc.vector.tensor_tensor(
                        att[:, 0:2, :], att[:, 0:2, :], att[:, 2:4, :],
                        Alu.add)
                    nc.vector.tensor_tensor(
                        att[:, 0, :], att[:, 0, :], att[:, 1, :], Alu.add)
                    if pending is not None:
                        finish(*pending)
                    pending = (h, att, psc, sl)
                finish(*pending)
                prev_ci = i

            # the last block's out projection has no next block to hide in
            for jb in range(JT):
                cblock_tile(prev_ci, jb)

    return nc


def _rope_tables():
    inv_freq = 1.0 / (10000.0 ** (np.arange(0, D, 2, dtype=np.float32) / D))
    t = np.arange(S, dtype=np.float32)
    freqs = np.einsum("s,f->sf", t, inv_freq)
    emb = np.concatenate([freqs, freqs], axis=-1)
    cosT = np.cos(emb).astype(np.float32).T.copy()
    sinT = np.sin(emb).astype(np.float32).T.copy()
    # fold the rotate-half sign in: out = qb*cos + halfswap(qb)*sinSW
    sinSW = np.concatenate([-sinT[:D // 2], sinT[D // 2:]], axis=0)
    return cosT.astype(BF), np.ascontiguousarray(sinSW).astype(BF)


def _core_inputs(x, Wqkv_w, Wqkv_b, out_w, b, g, cosT, sinT, xT_bf):
    # k-head columns first, then q-head columns (matches kernel layout)
    k_cols, q_cols, kb_rows, qb_rows = [], [], [], []
    for hl in range(HL):
        h = g * HL + hl
        q_cols.append(Wqkv_w[h * D:(h + 1) * D, :].T)
        k_cols.append(Wqkv_w[E + h * D:E + (h + 1) * D, :].T)
        qb_rows.append(Wqkv_b[h * D:(h + 1) * D])
        kb_rows.append(Wqkv_b[E + h * D:E + (h + 1) * D])
    wqkT = np.ascontiguousarray(
        np.concatenate(k_cols + q_cols, axis=1)).astype(BF)
    qkb = np.stack(kb_rows + qb_rows).astype(np.float32)
    v0 = 2 * E + g * HL * D
    wvT = np.ascontiguousarray(Wqkv_w[v0:v0 + HL * D, :].T).astype(BF)
    vb = Wqkv_b[v0:v0 + HL * D].astype(np.float32)
    owT = np.ascontiguousarray(
        out_w[:, g * HL * D:(g + 1) * HL * D].T).astype(BF)
    return {"xT": xT_bf, "wqkT": wqkT, "wvT": wvT, "qkb": qkb, "vb": vb,
            "cosT": cosT, "sinT": sinT, "owT": owT,
            "ones": np.ones((P, P), BF)}


def kernel(x, Wqkv_w, Wqkv_b, out_w, out_b):
    global LAST_EXEC_NS
    _install_axon_ntff_shim()
    from concourse.bass_utils import run_bass_kernel_spmd

    x = np.asarray(x, dtype=np.float32)
    Wqkv_w = np.asarray(Wqkv_w, dtype=np.float32)
    Wqkv_b = np.asarray(Wqkv_b, dtype=np.float32)
    out_w = np.asarray(out_w, dtype=np.float32)
    out_b = np.asarray(out_b, dtype=np.float32)

    cosT, sinT = _rope_tables()
    xT_bf = [np.ascontiguousarray(x[b].T).astype(BF) for b in range(2)]
    in_maps = []
    for core in range(8):
        b, g = core // 4, core % 4
        in_maps.append(
            _core_inputs(x, Wqkv_w, Wqkv_b, out_w, b, g, cosT, sinT, xT_bf[b]))

    nc = bass.Bass()
    _build_mha(nc)
    _split_multi_waits(nc)

    trace = bool(os.environ.get("MHA_TRACE"))
    if trace:
        # dev-only profiling path; skip the S3 artifact upload
        import concourse.bass_utils as _bu
        _bu.upload_artifacts = lambda tmpdir: tmpdir
    res = run_bass_kernel_spmd(
        nc, in_maps, core_ids=list(range(8)), trace=trace)
    if trace:
        LAST_EXEC_NS = res.exec_time_ns

    out = np.empty((2, S, E), dtype=np.float32)
    for b in range(2):
        acc = res.results[b * 4 + 0]["out"].astype(np.float32)
        for g in range(1, 4):
            acc += res.results[b * 4 + g]["out"].astype(np.float32)
        out[b] = acc + out_b[None, :]
    return out
